# revision 1
# baseline (speedup 1.0000x reference)
"""Trainium2 Bass kernel for nn_AnomalyDetector (LSTM encoder + 2x GCN + classifier).

Self-contained: kernel(**inputs) takes FULL numpy inputs, shards across 8
NeuronCores internally (data-parallel LSTM over nodes; node-partitioned GCN
with AllGather of transformed features and on-device edge-tile gathers),
returns the FULL [50000, 2] float32 softmax output.

Key design points:
- Feature-major LSTM (bf16 activations, fp8-e4m3 DoubleRow recurrent matmul,
  f32 cell state), 512-column chunks, PSUM-group-batched gate activations.
- GCN layers: per-core node shard; transformed features AllGathered into a
  node-major bf16 table; per-edge rows fetched with dma_gather (int16 indices,
  table split in two halves); aggregation via one-hot selection matmuls
  accumulated in PSUM per 128-node destination block, in a lo/hi pass with
  bf16 SBUF partial accumulation between passes.
"""
import numpy as np
import ml_dtypes

import concourse.bass as bass
import concourse.bacc as bacc
import concourse.mybir as mybir
import concourse.tile as tile
from concourse import bass_utils

BF16 = mybir.dt.bfloat16
F32 = mybir.dt.float32
I16 = mybir.dt.int16
FP8 = mybir.dt.float8e4
AF = mybir.ActivationFunctionType
ALU = mybir.AluOpType
PM = mybir.MatmulPerfMode
NPBF16 = ml_dtypes.bfloat16
NPFP8 = ml_dtypes.float8_e4m3

# problem constants (hardcoded per spec); configure() can override for tests
N, T, F, H, C = 50000, 32, 64, 256, 2
NCORES = 8
NS = N // NCORES
SP = 6272
NP = NCORES * SP
HALF = NP // 2
NBLK = SP // 128
CHUNKS = [512] * 12 + [128]
GROUPS = [(0, 2), (2, 2), (4, 2), (6, 2), (8, 2), (10, 2), (12, 1)]
PHASE = "full"   # "lstm" | "l1" | "full" — early-exit for bisection
DBG = None       # None | "h" | "h1" | "h2" — extra debug output of state tiles
G_MAX = 16       # max edge-tiles per dma_gather call

_CACHE = {}


def configure(n=None, t=None, sp=None, chunks=None, groups=None,
              phase=None, dbg=None, g_max=None):
    global N, T, NS, SP, NP, HALF, NBLK, CHUNKS, GROUPS, PHASE, DBG, G_MAX
    if n is not None:
        N = n
    if t is not None:
        T = t
    if sp is not None:
        SP = sp
    NS = N // NCORES
    NP = NCORES * SP
    HALF = NP // 2
    NBLK = SP // 128
    if chunks is not None:
        CHUNKS = chunks
    if groups is not None:
        GROUPS = groups
    if phase is not None:
        PHASE = phase
    if dbg is not None:
        DBG = dbg
    if g_max is not None:
        G_MAX = g_max
    _CACHE.clear()


def _pid(n):
    """original node id -> padded id"""
    return (n // NS) * SP + (n % NS)


def _wrap_idxs(lin):
    """dma_gather wrapped layout: [16, n/16] with wrapped[p, s] = lin[s*16+p]."""
    lin = np.asarray(lin)
    assert lin.size % 16 == 0
    return lin.reshape(-1, 16).T.astype(np.int16)


def _balanced_chunks(total, gmax):
    if total == 0:
        return []
    nc_ = -(-total // gmax)
    return [total // nc_ + (1 if i < total % nc_ else 0) for i in range(nc_)]


def _preprocess(x, edge_index, w_ih, w_hh, b_ih, b_hh, w1, b1, w2, b2, wc, bc):
    src = np.asarray(edge_index[0], np.int64)
    dst = np.asarray(edge_index[1], np.int64)

    deg = np.bincount(dst, minlength=N).astype(np.float64) + 1.0
    dinv = 1.0 / np.sqrt(deg)

    # edges + self loops, in padded ids
    ps = np.concatenate([_pid(src), _pid(np.arange(N))])
    pd = np.concatenate([_pid(dst), _pid(np.arange(N))])
    norm = np.concatenate([dinv[src] * dinv[dst], dinv * dinv]).astype(np.float32)

    core = pd // SP
    dloc = pd - core * SP
    blk = dloc // 128
    dblk = dloc % 128
    half = (ps >= HALF).astype(np.int64)
    ps_rel = ps - half * HALF

    # bucket edges per (core, half, block); sort by src within bucket
    order = np.lexsort((ps_rel, blk, half, core))
    ps_rel, dblk_s, norm_s = ps_rel[order], dblk[order], norm[order]
    core_s, blk_s, half_s = core[order], blk[order], half[order]
    key = (core_s * 2 + half_s) * NBLK + blk_s
    counts = np.bincount(key, minlength=NCORES * 2 * NBLK).reshape(NCORES, 2, NBLK)
    starts = np.zeros(NCORES * 2 * NBLK + 1, np.int64)
    np.cumsum(counts.reshape(-1), out=starts[1:])

    # common tile structure: per (half, block) -> tiles = ceil(max_core_count/128)
    cnt_max = counts.max(axis=0)                       # [2, NBLK]
    seg_tiles = -(-cnt_max // 128)                     # [2, NBLK]
    NT = int(seg_tiles.sum())

    # per-core edge-tile arrays; tile order: pass lo (blocks 0..), pass hi
    gidx = np.zeros((NCORES, 128, NT * 8), np.int16)
    dstloc_a = np.zeros((NCORES, 128, NT), NPBF16)
    enorm_a = np.zeros((NCORES, 128, NT), NPBF16)
    for c in range(NCORES):
        tt = 0
        for h in range(2):
            for b in range(NBLK):
                ntile = int(seg_tiles[h, b])
                if ntile == 0:
                    continue
                k = (c * 2 + h) * NBLK + b
                s0, s1 = starts[k], starts[k] + counts[c, h, b]
                ne = ntile * 128
                idx_l = np.zeros(ne, np.int64)
                dst_l = np.zeros(ne, np.int64)
                nrm_l = np.zeros(ne, np.float32)
                cnt = s1 - s0
                idx_l[:cnt] = ps_rel[s0:s1]
                dst_l[:cnt] = dblk_s[s0:s1]
                nrm_l[:cnt] = norm_s[s0:s1]
                gidx[c, :16, tt * 8:(tt + ntile) * 8] = _wrap_idxs(idx_l)
                dstloc_a[c, :, tt:tt + ntile] = dst_l.reshape(ntile, 128).T
                enorm_a[c, :, tt:tt + ntile] = nrm_l.reshape(ntile, 128).T.astype(NPBF16)
                tt += ntile
        gidx[c] = np.tile(gidx[c, :16], (8, 1))
    segs = [[int(seg_tiles[h, b]) for b in range(NBLK)] for h in range(2)]

    # weights / consts (shared across cores)
    bsum = (np.asarray(b_ih, np.float32) + np.asarray(b_hh, np.float32))
    consts = {
        "wiht": np.ascontiguousarray(np.asarray(w_ih, np.float32).T).astype(NPBF16),
        "whht": np.ascontiguousarray(np.asarray(w_hh, np.float32).T).astype(NPBF16),
        "bias": np.ascontiguousarray(bsum.reshape(8, 128).T),
        "w1": np.asarray(w1, np.float32).astype(NPBF16),
        "b1": np.ascontiguousarray(np.asarray(b1, np.float32).reshape(2, 128).T),
        "w2": np.asarray(w2, np.float32).astype(NPBF16),
        "b2": np.asarray(b2, np.float32).reshape(128, 1),
        "wc": np.asarray(wc, np.float32).astype(NPBF16),
        "bc": np.tile(np.asarray(bc, np.float32)[None, :], (128, 1)),
        "iota": np.tile(np.arange(128, dtype=NPBF16)[None, :], (128, 1)),
    }

    # x: per-core transposed [T, F, SP] bf16
    xf = np.asarray(x, np.float32)
    in_maps = []
    for c in range(NCORES):
        xs = xf[c * NS:(c + 1) * NS]                   # [NS, T, F]
        xt = np.zeros((T, F, SP), NPBF16)
        xt[:, :, :NS] = xs.transpose(1, 2, 0).astype(NPBF16)
        m = dict(consts)
        m["xt"] = xt
        m["gidx"] = gidx[c]
        m["dstloc"] = dstloc_a[c]
        m["enorm"] = enorm_a[c]
        in_maps.append(m)

    return in_maps, (NT, segs)


def _build(struct):
    NT, segs = struct
    nc = bacc.Bacc("TRN2", target_bir_lowering=False, debug=False,
                   num_devices=NCORES)

    xt = nc.dram_tensor("xt", [T, F, SP], BF16, kind="ExternalInput")
    wiht = nc.dram_tensor("wiht", [F, 4 * H], BF16, kind="ExternalInput")
    whht = nc.dram_tensor("whht", [H, 4 * H], BF16, kind="ExternalInput")
    bias = nc.dram_tensor("bias", [128, 8], F32, kind="ExternalInput")
    w1 = nc.dram_tensor("w1", [H, H], BF16, kind="ExternalInput")
    b1 = nc.dram_tensor("b1", [128, 2], F32, kind="ExternalInput")
    w2 = nc.dram_tensor("w2", [H, 128], BF16, kind="ExternalInput")
    b2 = nc.dram_tensor("b2", [128, 1], F32, kind="ExternalInput")
    wc = nc.dram_tensor("wc", [128, 2], BF16, kind="ExternalInput")
    bc = nc.dram_tensor("bc", [128, 2], F32, kind="ExternalInput")
    iota = nc.dram_tensor("iota", [128, 128], BF16, kind="ExternalInput")
    gidx = nc.dram_tensor("gidx", [128, NT * 8], I16, kind="ExternalInput")
    dstloc = nc.dram_tensor("dstloc", [128, NT], BF16, kind="ExternalInput")
    enorm = nc.dram_tensor("enorm", [128, NT], BF16, kind="ExternalInput")
    out = nc.dram_tensor("out", [SP, C], F32, kind="ExternalOutput")
    dbg = (nc.dram_tensor("dbg", [256, SP], F32, kind="ExternalOutput")
           if DBG else None)

    with tile.TileContext(nc) as tc:
        with tc.tile_pool(name="consts", bufs=1) as cp, \
             tc.tile_pool(name="hstate", bufs=1) as hp, \
             tc.tile_pool(name="dram", bufs=1, space="DRAM") as dp:

            wih_sb = cp.tile([F, 4 * H], BF16)
            nc.sync.dma_start(wih_sb[:], wiht[:])
            whh_sb = [cp.tile([128, 4 * H], BF16, name=f"whh{k}") for k in range(2)]
            for k in range(2):
                nc.sync.dma_start(whh_sb[k][:], whht[k * 128:(k + 1) * 128, :])
            bias_sb = cp.tile([128, 8], F32)
            nc.sync.dma_start(bias_sb[:], bias[:])
            w1_sb = [cp.tile([128, H], BF16, name=f"w1_{k}") for k in range(2)]
            for k in range(2):
                nc.sync.dma_start(w1_sb[k][:], w1[k * 128:(k + 1) * 128, :])
            b1_sb = cp.tile([128, 2], F32)
            nc.sync.dma_start(b1_sb[:], b1[:])
            w2_sb = [cp.tile([128, 128], BF16, name=f"w2_{k}") for k in range(2)]
            for k in range(2):
                nc.sync.dma_start(w2_sb[k][:], w2[k * 128:(k + 1) * 128, :])
            b2_sb = cp.tile([128, 1], F32)
            nc.sync.dma_start(b2_sb[:], b2[:])
            wc_sb = cp.tile([128, 2], BF16)
            nc.sync.dma_start(wc_sb[:], wc[:])
            bc_sb = cp.tile([128, 2], F32)
            nc.sync.dma_start(bc_sb[:], bc[:])
            iota_sb = cp.tile([128, 128], BF16)
            nc.sync.dma_start(iota_sb[:], iota[:])

            h = [hp.tile([128, SP], BF16, name=f"h{k}") for k in range(2)]

            # ================= LSTM =================
            with tc.tile_pool(name="lstm", bufs=2) as lp, \
                 tc.tile_pool(name="cstate", bufs=1) as cs, \
                 tc.tile_pool(name="lpsum", bufs=4, space="PSUM") as lps:
                cst = [cs.tile([128, SP], F32, name=f"c{k}") for k in range(2)]
                for k in range(2):
                    nc.vector.memset(h[k][:], 0)
                    nc.vector.memset(cst[k][:], 0)

                for t in range(T):
                  with nc.named_scope(f"lstm_t{t}"):
                    xsb = lp.tile([F, SP], BF16, tag="x")
                    nc.sync.dma_start(xsb[:], xt[t])
                    for (g0, gn) in GROUPS:
                        goff = g0 * 512
                        gsz = sum(CHUNKS[g0:g0 + gn])
                        gates = []
                        for m in range(8):
                            psum = lps.tile([128, 1024], F32, tag="gp")
                            mm = slice(m * 128, (m + 1) * 128)
                            for ki in range(3):
                                lhsT = (wih_sb[:, mm] if ki == 0
                                        else whh_sb[ki - 1][:, mm])
                                for ci in range(gn):
                                    csz = CHUNKS[g0 + ci]
                                    ssl = slice(goff + ci * 512, goff + ci * 512 + csz)
                                    psl = slice(ci * 512, ci * 512 + csz)
                                    rhs = (xsb[:, ssl] if ki == 0
                                           else h[ki - 1][:, ssl])
                                    nc.tensor.matmul(
                                        psum[:, psl], lhsT, rhs,
                                        start=(ki == 0), stop=(ki == 2))
                            func = AF.Tanh if m in (4, 5) else AF.Sigmoid
                            gt = lp.tile([128, 1024], BF16, tag=f"g{m}")
                            nc.scalar.activation(gt[:, :gsz], psum[:, :gsz], func,
                                                 bias=bias_sb[:, m:m + 1], scale=1.0)
                            gates.append(gt)
                        for k in range(2):
                            i_, f_, g_, o_ = (gates[k], gates[2 + k],
                                              gates[4 + k], gates[6 + k])
                            st = slice(goff, goff + gsz)
                            lo = slice(0, gsz)
                            nc.vector.tensor_tensor(
                                out=i_[:, lo], in0=i_[:, lo], in1=g_[:, lo], op=ALU.mult)
                            nc.vector.tensor_tensor(
                                out=cst[k][:, st], in0=f_[:, lo], in1=cst[k][:, st],
                                op=ALU.mult)
                            nc.vector.tensor_tensor(
                                out=cst[k][:, st], in0=cst[k][:, st], in1=i_[:, lo],
                                op=ALU.add)
                            nc.scalar.activation(g_[:, lo], cst[k][:, st], AF.Tanh)
                            nc.vector.tensor_tensor(
                                out=h[k][:, st], in0=o_[:, lo], in1=g_[:, lo],
                                op=ALU.mult)

            if DBG == "h":
                nc.gpsimd.dma_start(dbg[0:128, :], h[0][:])
                nc.gpsimd.dma_start(dbg[128:256, :], h[1][:])

            # ================= GCN =================
            if PHASE != "lstm":
              with tc.tile_pool(name="gcn", bufs=1) as gp, \
                 tc.tile_pool(name="gbuf", bufs=4) as gb, \
                 tc.tile_pool(name="gpsum", bufs=2, space="PSUM") as gps:
                gidx_sb = gp.tile([128, NT * 8], I16)
                nc.sync.dma_start(gidx_sb[:], gidx[:])
                dst_sb = gp.tile([128, NT], BF16)
                nc.sync.dma_start(dst_sb[:], dstloc[:])
                en_sb = gp.tile([128, NT], BF16)
                nc.sync.dma_start(en_sb[:], enorm[:])

                ag1_in = dp.tile([SP, H], BF16)
                table1 = dp.tile([NP, H], BF16, addr_space="Shared")
                ag2_in = dp.tile([SP, 128], BF16)
                table2 = dp.tile([NP, 128], BF16, addr_space="Shared")

                def transform_block(h_tiles, w_tiles, dout, ag_in, nb):
                    ps_ = gps.tile([128, dout], F32, tag="tf", name="ps_tf")
                    nsl = slice(nb * 128, (nb + 1) * 128)
                    nk = len(h_tiles)
                    for k in range(nk):
                        nc.tensor.matmul(ps_[:], h_tiles[k][:, nsl], w_tiles[k][:],
                                         start=(k == 0), stop=(k == nk - 1))
                    nm = gb.tile([128, dout], BF16, tag="nm", name="nm")
                    nc.vector.tensor_copy(out=nm[:], in_=ps_[:])
                    nc.sync.dma_start(ag_in[nsl, :], nm[:])

                def transform(h_tiles, w_tiles, dout, ag_in):
                    for nb in range(NBLK):
                        transform_block(h_tiles, w_tiles, dout, ag_in, nb)

                def aggregate(table, din, out_tiles, bias_aps, post_block=None):
                    nko = din // 128
                    tt = 0
                    for hh in range(2):
                        base = table[HALF:, :] if hh else table[:, :]
                        pass_tiles = sum(segs[hh])
                        csizes = _balanced_chunks(pass_tiles, G_MAX)
                        cb = [0]
                        for cz in csizes:
                            cb.append(cb[-1] + cz)
                        pass_t0 = tt
                        ci = 0
                        gout = None
                        cur_sel = None
                        for b in range(NBLK):
                            nt_b = segs[hh][b]
                            if nt_b == 0:
                                continue
                            psums = [gps.tile([128, 128], F32, tag=f"agg{j}",
                                              name=f"ps_agg{j}") for j in range(nko)]
                            for j in range(nt_b):
                                rel = tt - pass_t0
                                if rel == cb[ci]:
                                    ntile = csizes[ci]
                                    gout = gb.tile([128, G_MAX, din], BF16,
                                                   tag="gout", name="gout")
                                    nidx = ntile * 128
                                    nc.gpsimd.dma_gather(
                                        gout[:, :ntile, :], base,
                                        gidx_sb[:, tt * 8:(tt + ntile) * 8],
                                        nidx, nidx, din,
                                        single_packet=False)
                                    sel = gb.tile([128, G_MAX, 128], BF16,
                                                  tag="sel", name="sel")
                                    nc.vector.tensor_tensor(
                                        out=sel[:, :ntile, :],
                                        in0=dst_sb[:, tt:tt + ntile].unsqueeze(2)
                                            .broadcast_to([128, ntile, 128]),
                                        in1=iota_sb[:].unsqueeze(1)
                                            .broadcast_to([128, ntile, 128]),
                                        op=ALU.is_equal)
                                    nc.vector.tensor_tensor(
                                        out=sel[:, :ntile, :],
                                        in0=sel[:, :ntile, :],
                                        in1=en_sb[:, tt:tt + ntile].unsqueeze(2)
                                            .broadcast_to([128, ntile, 128]),
                                        op=ALU.mult)
                                    cur_sel = sel
                                    ci += 1
                                s = rel - cb[ci - 1]
                                for jk in range(nko):
                                    nc.tensor.matmul(
                                        psums[jk][:],
                                        gout[:, s, jk * 128:(jk + 1) * 128],
                                        cur_sel[:, s, :],
                                        start=(j == 0), stop=(j == nt_b - 1))
                                tt += 1
                            bsl = slice(b * 128, (b + 1) * 128)
                            if hh == 0:
                                for jk in range(nko):
                                    nc.vector.tensor_copy(
                                        out=out_tiles[jk][:, bsl], in_=psums[jk][:])
                            else:
                                for jk in range(nko):
                                    nc.vector.tensor_tensor(
                                        out=psums[jk][:], in0=psums[jk][:],
                                        in1=out_tiles[jk][:, bsl], op=ALU.add)
                                    nc.scalar.activation(
                                        out_tiles[jk][:, bsl], psums[jk][:],
                                        AF.Relu, bias=bias_aps[jk], scale=1.0)
                                if post_block is not None:
                                    post_block(b)

                h1t = [gp.tile([128, SP], BF16, name=f"h1t{k}") for k in range(2)]
                h2t = gp.tile([128, SP], BF16)

                with nc.named_scope("l1_tf"):
                    transform(h, w1_sb, H, ag1_in)
                with nc.named_scope("ag1"):
                    nc.gpsimd.collective_compute(
                        "AllGather", ALU.bypass,
                        replica_groups=[list(range(NCORES))],
                        ins=[ag1_in[:].opt()], outs=[table1[:].opt()])
                with nc.named_scope("l1_agg"):
                    aggregate(table1, H, h1t, [b1_sb[:, 0:1], b1_sb[:, 1:2]],
                              post_block=(None if PHASE == "l1" else (
                                  lambda b: transform_block(
                                      h1t, w2_sb, 128, ag2_in, b))))
                if DBG == "h1":
                    nc.gpsimd.dma_start(dbg[0:128, :], h1t[0][:])
                    nc.gpsimd.dma_start(dbg[128:256, :], h1t[1][:])

                if PHASE != "l1":
                  with nc.named_scope("ag2"):
                    nc.gpsimd.collective_compute(
                        "AllGather", ALU.bypass,
                        replica_groups=[list(range(NCORES))],
                        ins=[ag2_in[:].opt()], outs=[table2[:].opt()])
                  logit = gp.tile([128, NBLK, 2], F32)

                  def cls_block(nb):
                      psc = gps.tile([128, 2], F32, tag="cls", name="ps_cls")
                      nsl = slice(nb * 128, (nb + 1) * 128)
                      nc.tensor.matmul(psc[:], h2t[:, nsl], wc_sb[:],
                                       start=True, stop=True)
                      nc.vector.tensor_copy(out=logit[:, nb, :], in_=psc[:])
                  with nc.named_scope("l2_agg"):
                    aggregate(table2, 128, [h2t], [b2_sb[:, 0:1]],
                              post_block=cls_block)

                  # ---------- classifier + softmax ----------
                  l0 = logit[:, :, 0]
                  l1 = logit[:, :, 1]
                  nc.vector.tensor_scalar_add(out=l0, in0=l0, scalar1=bc_sb[:, 0:1])
                  nc.vector.tensor_scalar_add(out=l1, in0=l1, scalar1=bc_sb[:, 1:2])
                  mx = gp.tile([128, NBLK], F32)
                  nc.vector.tensor_tensor(out=mx[:], in0=l0, in1=l1, op=ALU.max)
                  nc.vector.tensor_tensor(out=l0, in0=l0, in1=mx[:], op=ALU.subtract)
                  nc.vector.tensor_tensor(out=l1, in0=l1, in1=mx[:], op=ALU.subtract)
                  e0 = gp.tile([128, NBLK], F32)
                  e1 = gp.tile([128, NBLK], F32)
                  nc.scalar.activation(e0[:], l0, AF.Exp)
                  nc.scalar.activation(e1[:], l1, AF.Exp)
                  nc.vector.tensor_tensor(out=mx[:], in0=e0[:], in1=e1[:], op=ALU.add)
                  nc.vector.reciprocal(mx[:], mx[:])
                  nc.vector.tensor_tensor(out=l0, in0=e0[:], in1=mx[:], op=ALU.mult)
                  nc.vector.tensor_tensor(out=l1, in0=e1[:], in1=mx[:], op=ALU.mult)
                  out_re = out[:].rearrange("(c p) j -> p c j", p=128)
                  nc.sync.dma_start(out_re, logit[:])
                  if DBG == "h2":
                      nc.gpsimd.dma_start(dbg[0:128, :], h2t[:])

    nc.compile()
    return nc


TRACE = False
LAST_EXEC_NS = None
LAST_DBG = None
LAST_RES = None


def kernel(**inputs):
    global LAST_EXEC_NS, LAST_DBG, LAST_RES
    in_maps, struct = _preprocess(**inputs)
    key = (NT_KEY := (struct[0], tuple(tuple(s) for s in struct[1])))
    if key not in _CACHE:
        _CACHE[key] = _build(struct)
    nc = _CACHE[key]
    kw = {}
    if TRACE:
        bass_utils.upload_artifacts = lambda tmpdir: "local://" + tmpdir
        kw["trace"] = True
    res = bass_utils.run_bass_kernel_spmd(
        nc, in_maps, core_ids=list(range(NCORES)), **kw)
    LAST_RES = res
    LAST_EXEC_NS = res.exec_time_ns
    if DBG:
        LAST_DBG = [res.results[c]["dbg"] for c in range(NCORES)]
    full = np.zeros((N, C), np.float32)
    for c in range(NCORES):
        full[c * NS:(c + 1) * NS] = res.results[c]["out"][:NS]
    return full



# revision 8
# speedup vs baseline: 1.0493x; 1.0493x over previous
"""Trainium2 Bass kernel for nn_AnomalyDetector (LSTM encoder + 2x GCN + classifier).

v2: quarter-pipelined. Nodes are split into 4 quarters per core; the LSTM
runs quarter-by-quarter and, as soon as a quarter's features are transformed
and AllGathered into a quarter-table, the GCN layer-1 edge gathers for
sources in that quarter run on GpSimd (dma_gather descriptor generation is
the serial bottleneck) CONCURRENTLY with the LSTM of later quarters.

Structural points vs v1:
- Aggregation sel matrices (one-hot dst x norm, [128 slots x 128 dst] per
  edge-tile) are precomputed on host and streamed via HWDGE DMA (no DVE
  is_equal/mult on device). The sel stream is shared by both GCN layers.
- Self-loops are not gathered: each dst-block gets one dense [128, F] tile
  DMA'd from the core's local (pre-AllGather) transform output, contracted
  against a host-built diag(dinv^2) sel tile.
- Both GCN layers aggregate in 4 quarter-passes accumulating via SBUF bf16.
- Gather indices are loaded per-chunk (tiny DMAs) instead of kept resident.

sel stream layout (index ts): for q in 0..3: [QB[q] self tiles, in block
order] then [sum(segs[q]) gathered tiles, in (block, tile) order].
gidx stream (index tg): for q: gathered tiles in (block, tile) order.
"""
import numpy as np
import ml_dtypes

import concourse.bass as bass
import concourse.bacc as bacc
import concourse.mybir as mybir
import concourse.tile as tile
from concourse import bass_utils

BF16 = mybir.dt.bfloat16
F32 = mybir.dt.float32
I16 = mybir.dt.int16
FP8 = mybir.dt.float8e4
AF = mybir.ActivationFunctionType
ALU = mybir.AluOpType
PM = mybir.MatmulPerfMode
NPBF16 = ml_dtypes.bfloat16
NPFP8 = ml_dtypes.float8_e4m3

# problem constants
N, T, F, H, C = 50000, 32, 64, 256, 2
NCORES = 8
NS = N // NCORES          # 6250 real nodes per core
SP = 6272                 # padded (49 blocks of 128)
NBLK = 49
NQ = 4
QB = [12, 12, 12, 13]                 # blocks per quarter
QOFF_B = [0, 12, 24, 36]
QW = [b * 128 for b in QB]            # column widths: 1536,1536,1536,1664
QOFF = [o * 128 for o in QOFF_B]
BLK2Q = np.repeat(np.arange(4), QB)   # block -> quarter
G_MAX = 16
PHASE = "full"   # "lstm" | "l1" | "full"

_CACHE = {}


def _wrap_idxs(lin):
    lin = np.asarray(lin)
    assert lin.size % 16 == 0
    return lin.reshape(-1, 16).T.astype(np.int16)


def _balanced_chunks(total, gmax):
    if total == 0:
        return []
    nc_ = -(-total // gmax)
    return [total // nc_ + (1 if i < total % nc_ else 0) for i in range(nc_)]


def _preprocess(x, edge_index, w_ih, w_hh, b_ih, b_hh, w1, b1, w2, b2, wc, bc):
    src = np.asarray(edge_index[0], np.int64)
    dst = np.asarray(edge_index[1], np.int64)

    deg = np.bincount(dst, minlength=N).astype(np.float64) + 1.0
    dinv = 1.0 / np.sqrt(deg)

    cs = src // NS
    ls = src % NS
    qs = BLK2Q[ls // 128]
    sidx = cs * np.asarray(QW)[qs] + (ls - np.asarray(QOFF)[qs])
    cd = dst // NS
    ld = dst % NS
    b = ld // 128
    dloc = ld % 128
    norm = (dinv[src] * dinv[dst]).astype(np.float32)

    order = np.lexsort((sidx, b, qs, cd))
    cd_s, qs_s, b_s = cd[order], qs[order], b[order]
    sidx_s, dloc_s, norm_s = sidx[order], dloc[order], norm[order]

    key = (cd_s * NQ + qs_s) * NBLK + b_s
    counts = np.bincount(key, minlength=NCORES * NQ * NBLK).reshape(
        NCORES, NQ, NBLK)
    starts = np.zeros(NCORES * NQ * NBLK + 1, np.int64)
    np.cumsum(counts.reshape(-1), out=starts[1:])

    cnt_max = counts.max(axis=0)                     # [NQ, NBLK]
    seg_tiles = -(-cnt_max // 128)
    assert (cnt_max > 0).all()
    NTG = int(seg_tiles.sum())
    NTS = NTG + NBLK
    segs = [[int(seg_tiles[q, bb]) for bb in range(NBLK)] for q in range(NQ)]

    gidx = np.zeros((NCORES, 128, NTG * 8), np.int16)
    sel = np.zeros((NCORES, 128, NTS, 128), NPBF16)
    for c in range(NCORES):
        tg = 0
        sbase = 0   # sel stream base of current quarter
        for q in range(NQ):
            # self tiles first (block order within quarter)
            for i, bb in enumerate(range(QOFF_B[q], QOFF_B[q] + QB[q])):
                nodes = c * NS + bb * 128 + np.arange(128)
                valid = (bb * 128 + np.arange(128)) < NS
                dv = np.where(valid, dinv[np.minimum(nodes, N - 1)] ** 2, 0.0)
                sel[c, np.arange(128), sbase + i, np.arange(128)] = \
                    dv.astype(NPBF16)
            gsel = sbase + QB[q]   # gathered sel tiles of this quarter
            toff = 0
            for bb in range(NBLK):
                ntile = int(seg_tiles[q, bb])
                k = (c * NQ + q) * NBLK + bb
                s0 = starts[k]
                cnt = counts[c, q, bb]
                idx_l = np.zeros(ntile * 128, np.int64)
                idx_l[:cnt] = sidx_s[s0:s0 + cnt]
                gidx[c, :16, tg * 8:(tg + ntile) * 8] = _wrap_idxs(idx_l)
                slot = np.arange(cnt)
                sel[c, slot % 128, gsel + toff + slot // 128,
                    dloc_s[s0:s0 + cnt]] = norm_s[s0:s0 + cnt].astype(NPBF16)
                tg += ntile
                toff += ntile
            sbase = gsel + toff
        gidx[c] = np.tile(gidx[c, :16], (8, 1))

    bsum = (np.asarray(b_ih, np.float32) + np.asarray(b_hh, np.float32))
    consts = {
        "wiht": np.ascontiguousarray(np.asarray(w_ih, np.float32).T
                                     ).astype(NPBF16),
        "whht": np.ascontiguousarray(np.asarray(w_hh, np.float32).T
                                     ).astype(NPBF16),
        "bias": np.ascontiguousarray(bsum.reshape(8, 128).T),
        "w1": np.asarray(w1, np.float32).astype(NPBF16),
        "b1": np.ascontiguousarray(np.asarray(b1, np.float32).reshape(2, 128).T),
        "w2": np.asarray(w2, np.float32).astype(NPBF16),
        "b2": np.asarray(b2, np.float32).reshape(128, 1),
        "wc": np.asarray(wc, np.float32).astype(NPBF16),
        "bc": np.tile(np.asarray(bc, np.float32)[None, :], (128, 1)),
    }

    xf = np.asarray(x, np.float32)
    in_maps = []
    for c in range(NCORES):
        xs = xf[c * NS:(c + 1) * NS]                   # [NS, T, F]
        xt = np.zeros((T, 128, SP), NPBF16)
        xt[:, :F, :NS] = xs.transpose(1, 2, 0).astype(NPBF16)
        xt[:, F:, :NS] = xt[:, :F, :NS]
        m = dict(consts)
        m["xt"] = xt
        m["gidx"] = gidx[c]
        m["sel"] = sel[c]
        in_maps.append(m)

    return in_maps, (NTG, NTS, tuple(tuple(s) for s in segs))


def _build(struct):
    NTG, NTS, segs = struct
    nc = bacc.Bacc("TRN2", target_bir_lowering=False, debug=False,
                   num_devices=NCORES)

    xt = nc.dram_tensor("xt", [T, 128, SP], BF16, kind="ExternalInput")
    wiht = nc.dram_tensor("wiht", [F, 4 * H], BF16, kind="ExternalInput")
    whht = nc.dram_tensor("whht", [H, 4 * H], BF16, kind="ExternalInput")
    bias = nc.dram_tensor("bias", [128, 8], F32, kind="ExternalInput")
    w1 = nc.dram_tensor("w1", [H, H], BF16, kind="ExternalInput")
    b1 = nc.dram_tensor("b1", [128, 2], F32, kind="ExternalInput")
    w2 = nc.dram_tensor("w2", [H, 128], BF16, kind="ExternalInput")
    b2 = nc.dram_tensor("b2", [128, 1], F32, kind="ExternalInput")
    wc = nc.dram_tensor("wc", [128, 2], BF16, kind="ExternalInput")
    bc = nc.dram_tensor("bc", [128, 2], F32, kind="ExternalInput")
    gidx = nc.dram_tensor("gidx", [128, NTG * 8], I16, kind="ExternalInput")
    seld = nc.dram_tensor("sel", [128, NTS, 128], BF16, kind="ExternalInput")
    out = nc.dram_tensor("out", [SP, C], F32, kind="ExternalOutput")

    # stream bases per quarter
    g_base = []     # gathered-tile (gidx) base per quarter
    s_self = []     # sel base of self tiles per quarter
    s_gat = []      # sel base of gathered tiles per quarter
    ts_ = tg_ = 0
    for q in range(NQ):
        s_self.append(ts_)
        s_gat.append(ts_ + QB[q])
        g_base.append(tg_)
        ts_ += QB[q] + sum(segs[q])
        tg_ += sum(segs[q])

    with tile.TileContext(nc) as tc:
        with tc.tile_pool(name="consts", bufs=1) as cp, \
             tc.tile_pool(name="state", bufs=1) as hp, \
             tc.tile_pool(name="dram", bufs=1, space="DRAM") as dp:

            wih_sb = cp.tile([F, 4 * H], BF16)
            nc.sync.dma_start(wih_sb[:], wiht[:])
            whh_sb = [cp.tile([128, 4 * H], BF16, name=f"whh{k}")
                      for k in range(2)]
            for k in range(2):
                nc.sync.dma_start(whh_sb[k][:], whht[k * 128:(k + 1) * 128, :])
            bias_sb = cp.tile([128, 8], F32)
            nc.sync.dma_start(bias_sb[:], bias[:])
            w1_sb = [cp.tile([128, H], BF16, name=f"w1_{k}") for k in range(2)]
            for k in range(2):
                nc.sync.dma_start(w1_sb[k][:], w1[k * 128:(k + 1) * 128, :])
            b1_sb = cp.tile([128, 2], F32)
            nc.sync.dma_start(b1_sb[:], b1[:])
            w2_sb = [cp.tile([128, 128], BF16, name=f"w2_{k}") for k in range(2)]
            for k in range(2):
                nc.sync.dma_start(w2_sb[k][:], w2[k * 128:(k + 1) * 128, :])
            b2_sb = cp.tile([128, 1], F32)
            nc.sync.dma_start(b2_sb[:], b2[:])
            wc_sb = cp.tile([128, 2], BF16)
            nc.sync.dma_start(wc_sb[:], wc[:])
            bc_sb = cp.tile([128, 2], F32)
            nc.sync.dma_start(bc_sb[:], bc[:])

            h = [hp.tile([128, SP], BF16, name=f"h{k}") for k in range(2)]
            cst = [hp.tile([128, SP], BF16, name=f"c{k}") for k in range(2)]
            h1t = [hp.tile([128, SP], BF16, name=f"h1t{k}") for k in range(2)]
            h2t = hp.tile([128, SP], BF16)
            logit = hp.tile([128, NBLK, 2], F32)
            for k in range(2):
                nc.vector.memset(h[k][:], 0)
                nc.vector.memset(cst[k][:], 0)

            ag1_in = [dp.tile([QW[q], H], BF16, name=f"ag1i{q}")
                      for q in range(NQ)]
            tab1 = [dp.tile([NCORES * QW[q], H], BF16, addr_space="Shared",
                            name=f"tab1{q}") for q in range(NQ)]
            ag2_in = [dp.tile([QW[q], 128], BF16, name=f"ag2i{q}")
                      for q in range(NQ)]
            tab2 = [dp.tile([NCORES * QW[q], 128], BF16, addr_space="Shared",
                            name=f"tab2{q}") for q in range(NQ)]

            with tc.tile_pool(name="work", bufs=2) as lp, \
                 tc.tile_pool(name="lpsum", bufs=2, space="PSUM") as lps, \
                 tc.tile_pool(name="gpsum", bufs=2, space="PSUM") as gps:

                def lstm_quarter(q):
                    """Generator: yields once per timestep."""
                    q0, qw = QOFF[q], QW[q]
                    groups = [(0, 1024), (1024, qw - 1024)]
                    for t in range(T):
                      yield
                      with nc.named_scope(f"lstm_q{q}_t{t}"):
                        xsb = lp.tile([128, 1664], BF16, tag="x")
                        nc.sync.dma_start(xsb[:F, :qw], xt[t, :F, q0:q0 + qw])
                        for (g0, gw) in groups:
                            gates = []
                            for m in range(8):
                                psum = lps.tile([128, 1024], F32, tag="gp")
                                mm = slice(m * 128, (m + 1) * 128)
                                for ki in range(3):
                                    lhsT = (wih_sb[:, mm] if ki == 0
                                            else whh_sb[ki - 1][:, mm])
                                    for c0 in range(0, gw, 512):
                                        csz = min(512, gw - c0)
                                        ssl = slice(q0 + g0 + c0,
                                                    q0 + g0 + c0 + csz)
                                        psl = slice(c0, c0 + csz)
                                        rhs = (xsb[:F, g0 + c0:g0 + c0 + csz]
                                               if ki == 0
                                               else h[ki - 1][:, ssl])
                                        nc.tensor.matmul(
                                            psum[:, psl], lhsT, rhs,
                                            start=(ki == 0), stop=(ki == 2))
                                func = AF.Tanh if m in (4, 5) else AF.Sigmoid
                                gt = lp.tile([128, 1024], BF16, tag=f"g{m}")
                                nc.scalar.activation(
                                    gt[:, :gw], psum[:, :gw], func,
                                    bias=bias_sb[:, m:m + 1], scale=1.0)
                                gates.append(gt)
                            for k in range(2):
                                i_, f_, g_, o_ = (gates[k], gates[2 + k],
                                                  gates[4 + k], gates[6 + k])
                                st = slice(q0 + g0, q0 + g0 + gw)
                                lo = slice(0, gw)
                                nc.vector.tensor_tensor(
                                    out=i_[:, lo], in0=i_[:, lo],
                                    in1=g_[:, lo], op=ALU.mult)
                                nc.vector.tensor_tensor(
                                    out=cst[k][:, st], in0=f_[:, lo],
                                    in1=cst[k][:, st], op=ALU.mult)
                                nc.vector.tensor_tensor(
                                    out=cst[k][:, st], in0=cst[k][:, st],
                                    in1=i_[:, lo], op=ALU.add)
                                nc.scalar.activation(g_[:, lo], cst[k][:, st],
                                                     AF.Tanh)
                                nc.vector.tensor_tensor(
                                    out=h[k][:, st], in0=o_[:, lo],
                                    in1=g_[:, lo], op=ALU.mult)

                def transform_block(h_tiles, w_tiles, dout, ag_in_q, bb, col0):
                    ps_ = gps.tile([128, 256], F32, tag="tf", name="ps_tf")
                    nsl = slice(bb * 128, (bb + 1) * 128)
                    nk = len(h_tiles)
                    for k in range(nk):
                        nc.tensor.matmul(ps_[:, :dout], h_tiles[k][:, nsl],
                                         w_tiles[k][:],
                                         start=(k == 0), stop=(k == nk - 1))
                    nm = lp.tile([128, 256], BF16, tag="nm", name="nm")
                    nc.vector.tensor_copy(out=nm[:, :dout], in_=ps_[:, :dout])
                    r0 = bb * 128 - col0
                    nc.sync.dma_start(ag_in_q[r0:r0 + 128, :], nm[:, :dout])

                def tf_quarter(q, h_tiles, w_tiles, dout, ag_in):
                    for bb in range(QOFF_B[q], QOFF_B[q] + QB[q]):
                        transform_block(h_tiles, w_tiles, dout, ag_in[q], bb,
                                        QOFF[q])

                def allgather(ag_in, tab, q, tag):
                    with nc.named_scope(f"ag{tag}_{q}"):
                        nc.gpsimd.collective_compute(
                            "AllGather", ALU.bypass,
                            replica_groups=[list(range(NCORES))],
                            ins=[ag_in[q][:].opt()], outs=[tab[q][:].opt()])

                def agg_pass(lay, q, tab_q, ag_in_q, din, out_tiles,
                             bias_aps, post_block):
                    """Generator: yields once per gather-chunk (for build-order
                    interleaving with LSTM timesteps)."""
                    nko = din // 128
                    pass_tiles = sum(segs[q])
                    csizes = _balanced_chunks(pass_tiles, G_MAX)
                    cb = [0]
                    for cz in csizes:
                        cb.append(cb[-1] + cz)
                    rel = 0
                    ci = 0
                    gout = None
                    sel_sb = None
                    first = (q == 0)
                    last = (q == NQ - 1)
                    nself_seen = 0
                    for bb in range(NBLK):
                        has_self = (BLK2Q[bb] == q)
                        ntile = segs[q][bb]
                        psum = gps.tile([128, 256], F32, tag="pb",
                                        name="ps_agg")
                        if has_self:
                            gself = lp.tile([128, 256], BF16, tag="gself")
                            r0 = bb * 128 - QOFF[q]
                            nc.sync.dma_start(gself[:, :din],
                                              ag_in_q[r0:r0 + 128, :])
                            sself = lp.tile([128, 128], BF16, tag="sself")
                            nc.sync.dma_start(
                                sself[:], seld[:, s_self[q] + nself_seen, :])
                            nself_seen += 1
                        nt_all = ntile + (1 if has_self else 0)
                        for jj in range(nt_all):
                            if has_self and jj == 0:
                                lhs = [gself[:, ko * 128:(ko + 1) * 128]
                                       for ko in range(nko)]
                                rhs = sself[:]
                            else:
                                if rel == cb[ci]:
                                    yield
                                    ctile = csizes[ci]
                                    tg0 = g_base[q] + cb[ci]
                                    gix = lp.tile([128, G_MAX * 8], I16,
                                                  tag="gix")
                                    nc.sync.dma_start(
                                        gix[:, :ctile * 8],
                                        gidx[:, tg0 * 8:(tg0 + ctile) * 8])
                                    gout = lp.tile([128, G_MAX, din], BF16,
                                                   tag=f"gout{lay}")
                                    nidx = ctile * 128
                                    nc.gpsimd.dma_gather(
                                        gout[:, :ctile, :], tab_q[:],
                                        gix[:, :ctile * 8],
                                        nidx, nidx, din,
                                        single_packet=False)
                                    sel_sb = lp.tile([128, G_MAX, 128], BF16,
                                                     tag="selsb")
                                    ss0 = s_gat[q] + cb[ci]
                                    nc.sync.dma_start(
                                        sel_sb[:, :ctile, :],
                                        seld[:, ss0:ss0 + ctile, :])
                                    ci += 1
                                s = rel - cb[ci - 1]
                                lhs = [gout[:, s, ko * 128:(ko + 1) * 128]
                                       for ko in range(nko)]
                                rhs = sel_sb[:, s, :]
                                rel += 1
                            for ko in range(nko):
                                nc.tensor.matmul(
                                    psum[:, ko * 128:(ko + 1) * 128],
                                    lhs[ko], rhs,
                                    start=(jj == 0), stop=(jj == nt_all - 1))
                        bsl = slice(bb * 128, (bb + 1) * 128)
                        if first:
                            for ko in range(nko):
                                nc.vector.tensor_copy(
                                    out=out_tiles[ko][:, bsl],
                                    in_=psum[:, ko * 128:(ko + 1) * 128])
                        elif not last:
                            for ko in range(nko):
                                nc.vector.tensor_tensor(
                                    out=out_tiles[ko][:, bsl],
                                    in0=psum[:, ko * 128:(ko + 1) * 128],
                                    in1=out_tiles[ko][:, bsl], op=ALU.add)
                        else:
                            for ko in range(nko):
                                nc.vector.tensor_tensor(
                                    out=psum[:, ko * 128:(ko + 1) * 128],
                                    in0=psum[:, ko * 128:(ko + 1) * 128],
                                    in1=out_tiles[ko][:, bsl], op=ALU.add)
                                nc.scalar.activation(
                                    out_tiles[ko][:, bsl],
                                    psum[:, ko * 128:(ko + 1) * 128],
                                    AF.Relu, bias=bias_aps[ko], scale=1.0)
                            if post_block is not None:
                                post_block(bb)

                def drain(gen):
                    for _ in gen:
                        pass

                def interleave(gen_a, gen_b):
                    """Alternate build steps: one LSTM timestep, one agg
                    chunk, so per-engine instruction order lets the gather
                    stream and the LSTM stream make progress concurrently."""
                    a_done = b_done = False
                    while not (a_done and b_done):
                        if not a_done:
                            try:
                                next(gen_a)
                            except StopIteration:
                                a_done = True
                        if not b_done:
                            try:
                                next(gen_b)
                            except StopIteration:
                                b_done = True

                # ================= schedule =================
                drain(lstm_quarter(0))
                tf_quarter(0, h, w1_sb, H, ag1_in)
                allgather(ag1_in, tab1, 0, "1")
                if PHASE == "lstm":
                    drain(lstm_quarter(1))
                    drain(lstm_quarter(2))
                    drain(lstm_quarter(3))
                else:
                    def tf2_post(bb):
                        q2 = int(BLK2Q[bb])
                        transform_block(h1t, w2_sb, 128, ag2_in[q2], bb,
                                        QOFF[q2])
                        if bb in (11, 23, 35, 48):
                            allgather(ag2_in, tab2, q2, "2")

                    with nc.named_scope("l1p0"):
                        interleave(lstm_quarter(1),
                                   agg_pass(1, 0, tab1[0], ag1_in[0], H, h1t,
                                            None, None))
                    tf_quarter(1, h, w1_sb, H, ag1_in)
                    allgather(ag1_in, tab1, 1, "1")
                    with nc.named_scope("l1p1"):
                        interleave(lstm_quarter(2),
                                   agg_pass(1, 1, tab1[1], ag1_in[1], H, h1t,
                                            None, None))
                    tf_quarter(2, h, w1_sb, H, ag1_in)
                    allgather(ag1_in, tab1, 2, "1")
                    with nc.named_scope("l1p2"):
                        interleave(lstm_quarter(3),
                                   agg_pass(1, 2, tab1[2], ag1_in[2], H, h1t,
                                            None, None))
                    tf_quarter(3, h, w1_sb, H, ag1_in)
                    allgather(ag1_in, tab1, 3, "1")
                    with nc.named_scope("l1p3"):
                        drain(agg_pass(1, 3, tab1[3], ag1_in[3], H, h1t,
                                       [b1_sb[:, 0:1], b1_sb[:, 1:2]],
                                       None if PHASE == "l1" else tf2_post))

                    if PHASE != "l1":
                        def cls_block(bb):
                            psc = gps.tile([128, 256], F32, tag="tf",
                                           name="ps_cls")
                            nsl = slice(bb * 128, (bb + 1) * 128)
                            nc.tensor.matmul(psc[:, 0:2], h2t[:, nsl],
                                             wc_sb[:], start=True, stop=True)
                            nc.vector.tensor_copy(out=logit[:, bb, :],
                                                  in_=psc[:, 0:2])

                        for q in range(NQ):
                            with nc.named_scope(f"l2p{q}"):
                                drain(agg_pass(2, q, tab2[q], ag2_in[q], 128,
                                               [h2t], [b2_sb[:, 0:1]],
                                               cls_block if q == NQ - 1
                                               else None))

                        l0 = logit[:, :, 0]
                        l1 = logit[:, :, 1]
                        nc.vector.tensor_scalar_add(out=l0, in0=l0,
                                                    scalar1=bc_sb[:, 0:1])
                        nc.vector.tensor_scalar_add(out=l1, in0=l1,
                                                    scalar1=bc_sb[:, 1:2])
                        mx = lp.tile([128, NBLK], F32, tag="sm0")
                        nc.vector.tensor_tensor(out=mx[:], in0=l0, in1=l1,
                                                op=ALU.max)
                        nc.vector.tensor_tensor(out=l0, in0=l0, in1=mx[:],
                                                op=ALU.subtract)
                        nc.vector.tensor_tensor(out=l1, in0=l1, in1=mx[:],
                                                op=ALU.subtract)
                        e0 = lp.tile([128, NBLK], F32, tag="sm1")
                        e1 = lp.tile([128, NBLK], F32, tag="sm2")
                        nc.scalar.activation(e0[:], l0, AF.Exp)
                        nc.scalar.activation(e1[:], l1, AF.Exp)
                        nc.vector.tensor_tensor(out=mx[:], in0=e0[:],
                                                in1=e1[:], op=ALU.add)
                        nc.vector.reciprocal(mx[:], mx[:])
                        nc.vector.tensor_tensor(out=l0, in0=e0[:], in1=mx[:],
                                                op=ALU.mult)
                        nc.vector.tensor_tensor(out=l1, in0=e1[:], in1=mx[:],
                                                op=ALU.mult)
                        out_re = out[:].rearrange("(c p) j -> p c j", p=128)
                        nc.sync.dma_start(out_re, logit[:])

    nc.compile()
    return nc


TRACE = False
LAST_EXEC_NS = None
LAST_RES = None


def kernel(**inputs):
    global LAST_EXEC_NS, LAST_RES
    in_maps, struct = _preprocess(**inputs)
    key = struct
    if key not in _CACHE:
        _CACHE[key] = _build(struct)
    nc = _CACHE[key]
    kw = {}
    if TRACE:
        bass_utils.upload_artifacts = lambda tmpdir: "local://" + tmpdir
        kw["trace"] = True
    res = bass_utils.run_bass_kernel_spmd(
        nc, in_maps, core_ids=list(range(NCORES)), **kw)
    LAST_RES = res
    LAST_EXEC_NS = res.exec_time_ns
    full = np.zeros((N, C), np.float32)
    for c in range(NCORES):
        full[c * NS:(c + 1) * NS] = res.results[c]["out"][:NS]
    return full
